# revision 2
# baseline (speedup 1.0000x reference)
"""Peephole Conv-LSTM (T=8,B=8,C=64,H=W=96,L=2,K=3) on 8 Trainium2 cores.

Strategy (v2)
-------------
Data-parallel over batch: core b processes batch item b through the whole
T x L recurrence (no collectives).

Conv-as-matmul: images live in SBUF as [C=64 partitions, 98*98 flat padded
pixels].  A 3x3 SAME conv is 9 shifted matmuls accumulating in PSUM, where
the shift is just a flat AP offset into the padded image.

Partition packing (K=128 contraction, M=128 outputs per matmul):
  * xh combo  [x_pad | h_pad]      -> conv(x,Wx*) + conv(h,Wh*) fuse into one
    K=128 matmul per tap; M packs two gates per pass.
  * cc combo  [c_pad | c_pad<<1]   -> peephole convs pair taps (dy,0)+(dy,1)
    in K; (dy,2) singletons are K=128 tiles with zeroed bottom rows.

v2 improvements over baseline:
  * per-chunk x prefetch of the NEXT step's input into xh[0:64] right after
    the last reader chunk of the current step -> no step-boundary PE bubble,
    PE stays HAM-warm (no 8us gaps, no cold-matmul ramps).
  * phase B (conv(c_new, Whc), M=64) pairs two chunks via column tiling
    (tile_position (0,0)/(0,64)) -> 2 concurrent M=64 matmuls, halving
    phase-B PE time.  The o-partial also stays resident in the phase-A
    PSUM tile (acto group continues into the whc matmuls, alternating
    [ct|o]/[o|ct] per chunk parity) -> no opart copy, no extra DVE add.
  * t=0 of each layer skips the peephole matmuls entirely (c_0 = 0) and
    computes c_new = ct*i.

v3 improvements:
  * K=64 "singleton" taps (dx=2 columns of the peephole/whc convs) run as
    row-tiled tile_position pairs: two K=64 matmuls execute concurrently in
    row groups (0,0)/(64,0), the upper group reading the +1-shift dup half.
    cif: 6 -> 5 PE slots; phase-B singles compose row+col tiling across the
    chunk pair: 3 -> 2 PE slots.
  * t=0 of each layer: h and c are zero, so gates need only the x convs.
    An [x | x<<1] dup (built piece-wise during the previous step) lets t=0
    use the same pair+single structure: 10 PE slots instead of 18 per chunk.
    At the step before a layer switch, h_new is written to a scratch tile
    and DMA'd per-piece straight to DRAM, freeing xh[64:128] for the dup.
"""

import numpy as np

import concourse.bass as bass
import concourse.mybir as mybir
from concourse import bacc
from concourse.bass_utils import run_bass_kernel_spmd
from concourse.tile import TileContext

F32 = mybir.dt.float32
F16 = mybir.dt.float16
AF = mybir.ActivationFunctionType

T, B, C, H, W, L, KS = 8, 8, 64, 96, 96, 2, 3
Hp = Wp = 98
NPIX = Hp * Wp              # 9604
ALLOC = NPIX + 1            # slack elem so +1-shifted dup reads stay in bounds
W0 = Wp + 1                 # flat index of output pixel (0,0) in padded coords
NWIN = 96 * Wp + 96 - W0 + 1  # 9406 flat positions spanning all output pixels
CH = 496
CHUNKS = [(W0 + i * CH, min(CH, NWIN - i * CH)) for i in range((NWIN + CH - 1) // CH)]
NCH = len(CHUNKS)
# prefetch piece boundaries: piece k = [PEND[k-1], PEND[k]); its last reader
# in the current step is phase_a(k+1), so it is issued right after that.
PEND = [min(W0 + k * CH + n + Wp + 1, ALLOC) for k, (_, n) in enumerate(CHUNKS)]
PEND[-1] = ALLOC

# row-tiled concurrent K=64 singles (tile_position row 64) — crashes the
# device on this stack (verified in isolation); keep off.
import os as _os
ROWTILE = _os.environ.get("ROWTILE", "0") == "1"

# profiling side-channel for test.py
LAST_EXEC_NS = None
LAST_RES = None


def _build(t_steps: int, n_layers: int, reps: int = 1) -> bass.Bass:
    nc = bacc.Bacc("TRN2", target_bir_lowering=False, debug=False,
                   enable_asserts=False, num_devices=8)

    xp = nc.declare_dram_parameter("xp", [t_steps, C, ALLOC], F16, isOutput=False)
    wd = {}
    wspec = [("aif", 9 * 128), ("cifP", 3 * 128), ("cifS", 3 * 128),
             ("actoE", 9 * 128), ("actoO", 9 * 128),
             ("whcP", 3 * 64), ("whcS", 3 * 64),
             ("aifT0P", 3 * 128), ("aifT0S", 3 * 128),
             ("actoT0PE", 3 * 128), ("actoT0SE", 3 * 128),
             ("actoT0PO", 3 * 128), ("actoT0SO", 3 * 128)]
    for l in range(n_layers):
        for nm, cols in wspec:
            wd[f"{nm}{l}"] = nc.declare_dram_parameter(
                f"{nm}{l}", [128, cols], F16, isOutput=False)
        for nm in ("bif", "bcD", "bobcD"):
            wd[f"{nm}{l}"] = nc.declare_dram_parameter(
                f"{nm}{l}", [128, 1], F32, isOutput=False)
    hs = nc.declare_dram_parameter("hs", [t_steps, C, ALLOC], F16, isOutput=True)
    cs = nc.declare_dram_parameter("cs", [t_steps, C, ALLOC], F16, isOutput=True)
    hscr = nc.dram_tensor("hscr", [t_steps, C, ALLOC], F16) if n_layers > 1 else None

    with TileContext(nc) as tc:
        with (
            tc.tile_pool(name="big", bufs=1) as bigp,
            tc.tile_pool(name="wp", bufs=1) as wp,
            tc.tile_pool(name="gp", bufs=1) as gp,
            tc.tile_pool(name="pif", bufs=2, space="PSUM") as pool_pif,
            tc.tile_pool(name="pcto", bufs=3, space="PSUM") as pool_pcto,
        ):
            xh = bigp.tile([128, ALLOC], F16, name="xh")
            cc0 = bigp.tile([128, ALLOC], F16, name="cc0")
            cc1 = bigp.tile([128, ALLOC], F16, name="cc1")

            wsb = {}
            for key, t_ in wd.items():
                rows = 128
                tl = wp.tile([rows, t_.shape[1]], t_.dtype, name=f"{key}sb")
                nc.sync.dma_start(tl[:], t_[:])
                wsb[key] = tl

            g_if = gp.tile([128, 2 * CH], F16, name="g_if")
            g_ct = gp.tile([64, 2 * CH], F16, name="g_ct")
            g_tmp = gp.tile([64, 2 * CH], F16, name="g_tmp")
            g_th = gp.tile([64, 2 * CH], F16, name="g_th")
            g_osb = gp.tile([64, 2 * CH], F16, name="g_osb")
            hout = gp.tile([64, 2 * CH + Wp], F16, name="hout")
            zscr = gp.tile([64, 128], F16, name="zscr")

            nc.vector.memset(xh[:], 0.0)
            nc.vector.memset(cc0[:], 0.0)
            nc.vector.memset(cc1[:], 0.0)
            nc.vector.memset(zscr[:], 0.0)
            nc.sync.dma_start(xh[0:64, :], xp[0, :, :])
            # [x | x<<1] dup for the t=0 pair/single conv structure
            nc.sync.dma_start(xh[64:128, 0:ALLOC - 1], xh[0:64, 1:ALLOC])

            import contextlib
            loop_ctx = tc.For_i(0, reps, 1) if reps > 1 else contextlib.nullcontext()
            step = 0
            with loop_ctx:
             for l in range(n_layers):
                for t in range(t_steps):
                    cur = cc0 if (step % 2 == 0) else cc1
                    nxt = cc1 if (step % 2 == 0) else cc0
                    first_t = (t == 0)
                    aifW = wsb[f"aif{l}"]
                    cifPW, cifSW = wsb[f"cifP{l}"], wsb[f"cifS{l}"]
                    actoEW, actoOW = wsb[f"actoE{l}"], wsb[f"actoO{l}"]
                    whcPW, whcSW = wsb[f"whcP{l}"], wsb[f"whcS{l}"]
                    aifT0P, aifT0S = wsb[f"aifT0P{l}"], wsb[f"aifT0S{l}"]
                    actoT0PE, actoT0SE = wsb[f"actoT0PE{l}"], wsb[f"actoT0SE{l}"]
                    actoT0PO, actoT0SO = wsb[f"actoT0PO{l}"], wsb[f"actoT0SO{l}"]
                    bifB = wsb[f"bif{l}"]
                    bcB = wsb[f"bcD{l}"]
                    bobcB = wsb[f"bobcD{l}"]

                    # source of the NEXT step's x-half (None on the last step)
                    if t + 1 < t_steps:
                        nsrc, nt = (xp if l == 0 else hscr), t + 1
                    elif l + 1 < n_layers:
                        nsrc, nt = hscr, 0
                    else:
                        nsrc, nt = None, 0
                    # next step is a layer's t=0: h_new is diverted to hout
                    # (freeing xh[64:128] for the [x|x<<1] dup of next layer)
                    pre_t0 = nsrc is not None and nt == 0

                    pcto_tiles = {}

                    def ring_zero(tile, p0, w0, n, base=None):
                        # zero pad-ring cols 0/97 garbage inside flat [w0, w0+n)
                        # base!=None: tile is chunk-local, cols at base+off-w0
                        for rem in (0, Wp - 1):
                            first = ((w0 - rem + Wp - 1) // Wp) * Wp + rem
                            if first >= w0 + n:
                                continue
                            cnt = (w0 + n - 1 - first) // Wp + 1
                            c0 = first if base is None else base + first - w0
                            v = tile[p0:p0 + 64, c0:c0 + Wp * cnt]
                            v = v.rearrange("p (r w) -> p r w", w=Wp)
                            nc.vector.memset(v[:, :, 0:1], 0.0)

                    def pair_single_group(psum, pw, sw, src, w0, n, start_grp):
                        # 3 K=128 tap-pair matmuls [(dy,0)+(dy,1)] plus 3
                        # K=128 singles [(dy,2); zero rows] — full-K matmuls
                        # only (sub-K matmuls measure ~15% slower here).
                        for dy in range(3):
                            s = w0 + (dy - 1) * Wp - 1
                            nc.tensor.matmul(
                                psum[:, :n], pw[:, dy * 128:(dy + 1) * 128],
                                src[:, s:s + n],
                                start=(start_grp and dy == 0), stop=False)
                        for dy in range(3):
                            s2 = w0 + (dy - 1) * Wp + 1
                            nc.tensor.matmul(
                                psum[:, :n], sw[:, dy * 128:(dy + 1) * 128],
                                src[:, s2:s2 + n],
                                start=False, stop=(dy == 2))

                    def phase_a(k):
                        w0, n = CHUNKS[k]
                        even = (k % 2 == 0)
                        p_if = pool_pif.tile([128, CH], F32, tag="pif")
                        p_cto = pool_pcto.tile([128, CH], F32, tag="pcto")
                        pcto_tiles[k] = p_cto
                        if first_t:
                            # h = c = 0: only the x convs, via [x | x<<1]
                            pair_single_group(p_if, aifT0P, aifT0S, xh, w0, n,
                                              True)
                            pT0 = actoT0PE if even else actoT0PO
                            sT0 = actoT0SE if even else actoT0SO
                            pair_single_group(p_cto, pT0, sT0, xh, w0, n, True)
                        else:
                            for j in range(9):
                                dy, dx = divmod(j, 3)
                                s = w0 + (dy - 1) * Wp + (dx - 1)
                                nc.tensor.matmul(
                                    p_if[:, :n],
                                    aifW[:, j * 128:(j + 1) * 128],
                                    xh[:, s:s + n],
                                    start=(j == 0), stop=False)
                            pair_single_group(p_if, cifPW, cifSW, cur, w0, n,
                                              False)
                            actoW = actoEW if even else actoOW
                            for j in range(9):
                                dy, dx = divmod(j, 3)
                                s = w0 + (dy - 1) * Wp + (dx - 1)
                                nc.tensor.matmul(
                                    p_cto[:, :n],
                                    actoW[:, j * 128:(j + 1) * 128],
                                    xh[:, s:s + n],
                                    start=(j == 0), stop=(j == 8))

                        sl = slice((k % 2) * CH, (k % 2) * CH + n)
                        wsl = slice(w0, w0 + n)
                        ct_in = p_cto[0:64, :n] if even else p_cto[64:128, :n]
                        bc_sl = bcB[0:64] if even else bcB[64:128]
                        nc.scalar.activation(g_if[:, sl], p_if[:, :n], AF.Sigmoid,
                                             bias=bifB[:])
                        nc.scalar.activation(g_ct[:, sl], ct_in, AF.Tanh,
                                             bias=bc_sl)
                        if first_t:
                            # c_new = ct * i   (c_prev == 0)
                            nc.vector.tensor_mul(nxt[0:64, wsl], g_ct[:, sl],
                                                 g_if[0:64, sl])
                        else:
                            # c_new = c*f + ct*i ; read c via the +1-shift dup
                            # half so both inputs share start partition 64
                            nc.vector.tensor_mul(nxt[0:64, wsl],
                                                 cur[64:128, w0 - 1:w0 - 1 + n],
                                                 g_if[64:128, sl])
                            nc.vector.tensor_mul(g_tmp[:, sl], g_ct[:, sl],
                                                 g_if[0:64, sl])
                            nc.vector.tensor_add(nxt[0:64, wsl],
                                                 nxt[0:64, wsl],
                                                 g_tmp[:, sl])
                        # ring-zero this chunk of c_new, then its dup stripe
                        ring_zero(nxt, 0, w0, n)
                        nc.sync.dma_start(nxt[64:128, w0 - 1:w0 - 1 + n],
                                          nxt[0:64, w0:w0 + n])

                    def pb_mm(k, row, wcols, s_off, stop, from_dup, sw=False):
                        # one whc matmul into this chunk's o-half of p_cto
                        w0, n = CHUNKS[k]
                        col = 64 if (k % 2 == 0) else 0
                        p_cto = pcto_tiles[k]
                        s = w0 + s_off - (1 if from_dup else 0)
                        wt = whcSW if sw else whcPW
                        nc.tensor.matmul(
                            p_cto[col:col + 64, :n],
                            wt[:, wcols[0]:wcols[1]], nxt[:, s:s + n],
                            start=False, stop=stop,
                            tile_position=(0, col),
                            skip_group_check=True)

                    def phase_b_post(k):
                        w0, n = CHUNKS[k]
                        even = (k % 2 == 0)
                        col = 64 if even else 0
                        p_cto = pcto_tiles.pop(k)
                        sl = slice((k % 2) * CH, (k % 2) * CH + n)
                        wsl = slice(w0, w0 + n)
                        bo_sl = bobcB[col:col + 64]
                        nc.scalar.activation(g_osb[:, sl], p_cto[col:col + 64, :n],
                                             AF.Sigmoid, bias=bo_sl)
                        nc.scalar.activation(g_th[:, sl], nxt[0:64, wsl], AF.Tanh)
                        if pre_t0:
                            # divert h_new to hout and stream it to DRAM
                            nc.vector.tensor_mul(hout[:, sl], g_osb[:, sl],
                                                 g_th[:, sl])
                            ring_zero(hout, 0, w0, n, base=(k % 2) * CH)
                            nc.sync.dma_start(nsrc[t, :, w0:w0 + n],
                                              hout[:, sl])
                        else:
                            nc.vector.tensor_mul(xh[64:128, wsl], g_osb[:, sl],
                                                 g_th[:, sl])
                            ring_zero(xh, 64, w0, n)

                    def phase_b_pair(ka, kb):
                        ks = [ka] if kb is None else [ka, kb]
                        # K=128 tap-pair matmuls, column-tiled across the pair
                        for dy in range(3):
                            for k in ks:
                                pb_mm(k, -1, (dy * 64, (dy + 1) * 64),
                                      (dy - 1) * Wp - 1, False, False)
                        # K=128 singles [(dy,2); zero rows], col-tiled
                        for dy in range(3):
                            for k in ks:
                                pb_mm(k, -1, (dy * 64, (dy + 1) * 64),
                                      (dy - 1) * Wp + 1, dy == 2, False,
                                      sw=True)
                        for k in ks:
                            phase_b_post(k)

                    def xpiece(lo, hi):
                        nc.sync.dma_start(xh[0:64, lo:hi], nsrc[nt, :, lo:hi])
                        if pre_t0:
                            dlo = max(lo - 1, 0)
                            nc.sync.dma_start(xh[64:128, dlo:hi - 1],
                                              xh[0:64, dlo + 1:hi])

                    for k in range(NCH):
                        phase_a(k)
                        if nsrc is not None and k >= 1:
                            xpiece(PEND[k - 2] if k >= 2 else 0, PEND[k - 1])
                        if k >= 2 and k % 2 == 0:
                            phase_b_pair(k - 2, k - 1)
                    if nsrc is not None:
                        xpiece(PEND[NCH - 2], ALLOC)
                    phase_b_pair(NCH - 1, None)
                    if first_t and not pre_t0:
                        # the [x|x<<1] dup left x[W0] at flat 98 (= ring col 0
                        # of row 1, below every chunk window); it must be zero
                        # when this half is next read/saved as h
                        nc.vector.memset(xh[64:128, Wp:Wp + 1], 0.0)

                    if pre_t0:
                        # body pieces went out in phase_b_post; pad head/tail
                        nc.sync.dma_start(nsrc[t, :, 0:W0], zscr[:, 0:W0])
                        tail = CHUNKS[-1][0] + CHUNKS[-1][1]
                        nc.sync.dma_start(nsrc[t, :, tail:ALLOC],
                                          zscr[:, 0:ALLOC - tail])
                    elif l == 0 and n_layers > 1:
                        nc.sync.dma_start(hscr[t, :, :], xh[64:128, :])
                    if l == n_layers - 1:
                        nc.sync.dma_start(hs[t, :, :], xh[64:128, :])
                        nc.sync.dma_start(cs[t, :, :], nxt[0:64, :])
                    step += 1
    nc.compile()
    return nc


def _pack_weights(l, wxi, whi, wci, wxf, whf, wcf, wxc, whc, wxo, who, wco,
                  b_i, b_f, b_c, b_o):
    Z64 = np.zeros((64, 64), np.float32)
    Z128 = np.zeros((64, 128), np.float32)

    def tap(wa, wb, dy, dx):
        # [128(k), 128(m)] block: k = x_cin | h_cin, m = gateA | gateB c_out
        top = np.concatenate([wa[0][l, :, :, dy, dx].T, wb[0][l, :, :, dy, dx].T],
                             axis=1)
        bot = np.concatenate([wa[1][l, :, :, dy, dx].T, wb[1][l, :, :, dy, dx].T],
                             axis=1)
        return np.concatenate([top, bot], axis=0)

    def group(wa, wb):
        return np.concatenate([tap(wa, wb, dy, dx)
                               for dy in range(3) for dx in range(3)], axis=1)

    def m128(wa, wb, dy, dx):
        # x-part-only [64, 128] block
        return np.concatenate([wa[l, :, :, dy, dx].T, wb[l, :, :, dy, dx].T],
                              axis=1)

    def ps_pack(blk, m):
        # pair tiles [(dy,0) ; (dy,1)] for dy=0..2, then singles tile
        # [[ (0,2) | (1,2) | (2,2) ] ; [ 0 | (1,2) | 0 ]]  (m = 64/128 cols)
        z = Z64 if m == 64 else Z128
        pairs = np.concatenate(
            [np.concatenate([blk(dy, 0), blk(dy, 1)], axis=0)
             for dy in range(3)], axis=1)
        singles = np.concatenate([
            np.concatenate([blk(0, 2), z], axis=0),
            np.concatenate([blk(1, 2), z], axis=0),
            np.concatenate([blk(2, 2), z], axis=0)], axis=1)
        return pairs, singles

    cifP, cifS = ps_pack(lambda dy, dx: m128(wci, wcf, dy, dx), 128)
    whcP, whcS = ps_pack(lambda dy, dx: whc[l, :, :, dy, dx].T, 64)
    aifT0P, aifT0S = ps_pack(lambda dy, dx: m128(wxi, wxf, dy, dx), 128)
    actoT0PE, actoT0SE = ps_pack(lambda dy, dx: m128(wxc, wxo, dy, dx), 128)
    actoT0PO, actoT0SO = ps_pack(lambda dy, dx: m128(wxo, wxc, dy, dx), 128)

    def pad128(v):
        # weight dram tensors are declared [128, cols]
        if v.shape[0] == 128:
            return v
        return np.concatenate([v, np.zeros((128 - v.shape[0], v.shape[1]),
                                           np.float32)], axis=0)

    out = {
        f"aif{l}": group((wxi, whi), (wxf, whf)),
        f"cifP{l}": cifP, f"cifS{l}": cifS,
        f"actoE{l}": group((wxc, whc), (wxo, who)),
        f"actoO{l}": group((wxo, who), (wxc, whc)),
        f"whcP{l}": whcP, f"whcS{l}": whcS,
        f"aifT0P{l}": aifT0P, f"aifT0S{l}": aifT0S,
        f"actoT0PE{l}": actoT0PE, f"actoT0SE{l}": actoT0SE,
        f"actoT0PO{l}": actoT0PO, f"actoT0SO{l}": actoT0SO,
        f"bif{l}": np.concatenate([b_i[l], b_f[l]]).reshape(128, 1).astype(np.float32),
        f"bcD{l}": np.concatenate([b_c[l], b_c[l]]).reshape(128, 1).astype(np.float32),
        f"bobcD{l}": np.concatenate([b_o[l] + b_c[l], b_o[l] + b_c[l]]
                                    ).reshape(128, 1).astype(np.float32),
    }
    return {k: (np.ascontiguousarray(v, np.float32) if k.startswith("b")
                else np.ascontiguousarray(pad128(v), np.float16))
            for k, v in out.items()}


def kernel(x, wxi, whi, wci, wxf, whf, wcf, wxc, whc, wxo, who, wco,
           b_i, b_f, b_c, b_o):
    global LAST_EXEC_NS, LAST_RES
    t_steps, bsz = x.shape[0], x.shape[1]
    assert (t_steps, bsz) == (T, B)

    wmaps = {}
    for l in range(L):
        wmaps.update(_pack_weights(l, wxi, whi, wci, wxf, whf, wcf, wxc, whc,
                                   wxo, who, wco, b_i, b_f, b_c, b_o))

    # pad x per batch item: [B, T, C, ALLOC]
    xp = np.zeros((B, T, C, ALLOC), np.float16)
    xview = xp[:, :, :, :NPIX].reshape(B, T, C, Hp, Wp)
    xview[:, :, :, 1:97, 1:97] = np.transpose(x, (1, 0, 2, 3, 4))

    nc = _build(T, L)
    in_maps = [dict(wmaps, xp=np.ascontiguousarray(xp[b])) for b in range(B)]
    import os
    res = run_bass_kernel_spmd(nc, in_maps, core_ids=list(range(B)))
    LAST_EXEC_NS = res.exec_time_ns
    LAST_RES = res

    hs = np.zeros((T, B, C, H, W), np.float32)
    cs = np.zeros((T, B, C, H, W), np.float32)
    for b in range(B):
        hp = res.results[b]["hs"][:, :, :NPIX].astype(np.float32).reshape(T, C, Hp, Wp)
        cp = res.results[b]["cs"][:, :, :NPIX].astype(np.float32).reshape(T, C, Hp, Wp)
        hs[:, b] = hp[:, :, 1:97, 1:97]
        cs[:, b] = cp[:, :, 1:97, 1:97]
    return np.stack([hs, cs])


# revision 4
# speedup vs baseline: 1.0722x; 1.0722x over previous
"""Peephole Conv-LSTM (T=8,B=8,C=64,H=W=96,L=2,K=3) on 8 Trainium2 cores.

Strategy (v2)
-------------
Data-parallel over batch: core b processes batch item b through the whole
T x L recurrence (no collectives).

Conv-as-matmul: images live in SBUF as [C=64 partitions, 98*98 flat padded
pixels].  A 3x3 SAME conv is 9 shifted matmuls accumulating in PSUM, where
the shift is just a flat AP offset into the padded image.

Partition packing (K=128 contraction, M=128 outputs per matmul):
  * xh combo  [x_pad | h_pad]      -> conv(x,Wx*) + conv(h,Wh*) fuse into one
    K=128 matmul per tap; M packs two gates per pass.
  * cc combo  [c_pad | c_pad<<1]   -> peephole convs pair taps (dy,0)+(dy,1)
    in K; (dy,2) singletons are K=128 tiles with zeroed bottom rows.

v2 improvements over baseline:
  * per-chunk x prefetch of the NEXT step's input into xh[0:64] right after
    the last reader chunk of the current step -> no step-boundary PE bubble,
    PE stays HAM-warm (no 8us gaps, no cold-matmul ramps).
  * phase B (conv(c_new, Whc), M=64) pairs two chunks via column tiling
    (tile_position (0,0)/(0,64)) -> 2 concurrent M=64 matmuls, halving
    phase-B PE time.  The o-partial also stays resident in the phase-A
    PSUM tile (acto group continues into the whc matmuls, alternating
    [ct|o]/[o|ct] per chunk parity) -> no opart copy, no extra DVE add.
  * t=0 of each layer skips the peephole matmuls entirely (c_0 = 0) and
    computes c_new = ct*i.

v3 improvements:
  * K=64 "singleton" taps (dx=2 columns of the peephole/whc convs) run as
    row-tiled tile_position pairs: two K=64 matmuls execute concurrently in
    row groups (0,0)/(64,0), the upper group reading the +1-shift dup half.
    cif: 6 -> 5 PE slots; phase-B singles compose row+col tiling across the
    chunk pair: 3 -> 2 PE slots.
  * t=0 of each layer: h and c are zero, so gates need only the x convs.
    An [x | x<<1] dup (built piece-wise during the previous step) lets t=0
    use the same pair+single structure: 10 PE slots instead of 18 per chunk.
    At the step before a layer switch, h_new is written to a scratch tile
    and DMA'd per-piece straight to DRAM, freeing xh[64:128] for the dup.
"""

import numpy as np

import concourse.bass as bass
import concourse.mybir as mybir
from concourse import bacc
from concourse.bass_utils import run_bass_kernel_spmd
from concourse.tile import TileContext

F32 = mybir.dt.float32
F16 = mybir.dt.float16
AF = mybir.ActivationFunctionType

T, B, C, H, W, L, KS = 8, 8, 64, 96, 96, 2, 3
Hp = Wp = 98
NPIX = Hp * Wp              # 9604
ALLOC = NPIX + 1            # slack elem so +1-shifted dup reads stay in bounds
W0 = Wp + 1                 # flat index of output pixel (0,0) in padded coords
NWIN = 96 * Wp + 96 - W0 + 1  # 9406 flat positions spanning all output pixels
CH = 496
CHUNKS = [(W0 + i * CH, min(CH, NWIN - i * CH)) for i in range((NWIN + CH - 1) // CH)]
NCH = len(CHUNKS)
# prefetch piece boundaries: piece k = [PEND[k-1], PEND[k]); its last reader
# in the current step is phase_a(k+1), so it is issued right after that.
PEND = [min(W0 + k * CH + n + Wp + 1, ALLOC) for k, (_, n) in enumerate(CHUNKS)]
PEND[-1] = ALLOC

# row-tiled concurrent K=64 singles (tile_position row 64) — crashes the
# device on this stack (verified in isolation); keep off.
import os as _os
ROWTILE = _os.environ.get("ROWTILE", "0") == "1"

# profiling side-channel for test.py
LAST_EXEC_NS = None
LAST_RES = None


def _build(t_steps: int, n_layers: int, reps: int = 1) -> bass.Bass:
    nc = bacc.Bacc("TRN2", target_bir_lowering=False, debug=False,
                   enable_asserts=False, num_devices=8)

    xp = nc.declare_dram_parameter("xp", [t_steps, C, ALLOC], F16, isOutput=False)
    wd = {}
    wspec = [("aif", 9 * 128), ("cifP", 3 * 128), ("cifS", 3 * 128),
             ("actoE", 9 * 128), ("actoO", 9 * 128),
             ("whcP", 3 * 64), ("whcS", 3 * 64),
             ("aifT0P", 3 * 128), ("aifT0S", 3 * 128),
             ("actoT0PE", 3 * 128), ("actoT0SE", 3 * 128),
             ("actoT0PO", 3 * 128), ("actoT0SO", 3 * 128)]
    for l in range(n_layers):
        for nm, cols in wspec:
            wd[f"{nm}{l}"] = nc.declare_dram_parameter(
                f"{nm}{l}", [128, cols], F16, isOutput=False)
        for nm in ("bif", "bcD", "bobcD"):
            wd[f"{nm}{l}"] = nc.declare_dram_parameter(
                f"{nm}{l}", [128, 1], F32, isOutput=False)
    hs = nc.declare_dram_parameter("hs", [t_steps, C, ALLOC], F16, isOutput=True)
    cs = nc.declare_dram_parameter("cs", [t_steps, C, ALLOC], F16, isOutput=True)
    hscr = nc.dram_tensor("hscr", [t_steps, C, ALLOC], F16) if n_layers > 1 else None

    with TileContext(nc) as tc:
        with (
            tc.tile_pool(name="big", bufs=1) as bigp,
            tc.tile_pool(name="wp", bufs=1) as wp,
            tc.tile_pool(name="gp", bufs=1) as gp,
            tc.tile_pool(name="pif", bufs=2, space="PSUM") as pool_pif,
            tc.tile_pool(name="pcto", bufs=4, space="PSUM") as pool_pcto,
        ):
            xh = bigp.tile([128, ALLOC], F16, name="xh")
            cc0 = bigp.tile([128, ALLOC], F16, name="cc0")
            cc1 = bigp.tile([128, ALLOC], F16, name="cc1")

            wsb = {}
            for key, t_ in wd.items():
                rows = 128
                tl = wp.tile([rows, t_.shape[1]], t_.dtype, name=f"{key}sb")
                nc.sync.dma_start(tl[:], t_[:])
                wsb[key] = tl

            g_if = gp.tile([128, 2 * CH], F16, name="g_if")
            g_ct = gp.tile([64, 2 * CH], F16, name="g_ct")
            g_tmp = gp.tile([64, 2 * CH], F16, name="g_tmp")
            g_th = gp.tile([64, 2 * CH], F16, name="g_th")
            g_osb = gp.tile([64, 2 * CH], F16, name="g_osb")
            hout = gp.tile([64, 2 * CH + Wp], F16, name="hout")
            zscr = gp.tile([64, 128], F16, name="zscr")

            nc.vector.memset(xh[:], 0.0)
            nc.vector.memset(cc0[:], 0.0)
            nc.vector.memset(cc1[:], 0.0)
            nc.vector.memset(zscr[:], 0.0)
            nc.sync.dma_start(xh[0:64, :], xp[0, :, :])
            # [x | x<<1] dup for the t=0 pair/single conv structure
            nc.sync.dma_start(xh[64:128, 0:ALLOC - 1], xh[0:64, 1:ALLOC])

            import contextlib
            loop_ctx = tc.For_i(0, reps, 1) if reps > 1 else contextlib.nullcontext()
            step = 0
            with loop_ctx:
             for l in range(n_layers):
                for t in range(t_steps):
                    cur = cc0 if (step % 2 == 0) else cc1
                    nxt = cc1 if (step % 2 == 0) else cc0
                    first_t = (t == 0)
                    aifW = wsb[f"aif{l}"]
                    cifPW, cifSW = wsb[f"cifP{l}"], wsb[f"cifS{l}"]
                    actoEW, actoOW = wsb[f"actoE{l}"], wsb[f"actoO{l}"]
                    whcPW, whcSW = wsb[f"whcP{l}"], wsb[f"whcS{l}"]
                    aifT0P, aifT0S = wsb[f"aifT0P{l}"], wsb[f"aifT0S{l}"]
                    actoT0PE, actoT0SE = wsb[f"actoT0PE{l}"], wsb[f"actoT0SE{l}"]
                    actoT0PO, actoT0SO = wsb[f"actoT0PO{l}"], wsb[f"actoT0SO{l}"]
                    bifB = wsb[f"bif{l}"]
                    bcB = wsb[f"bcD{l}"]
                    bobcB = wsb[f"bobcD{l}"]

                    # source of the NEXT step's x-half (None on the last step)
                    if t + 1 < t_steps:
                        nsrc, nt = (xp if l == 0 else hscr), t + 1
                    elif l + 1 < n_layers:
                        nsrc, nt = hscr, 0
                    else:
                        nsrc, nt = None, 0
                    # next step is a layer's t=0: h_new is diverted to hout
                    # (freeing xh[64:128] for the [x|x<<1] dup of next layer)
                    pre_t0 = nsrc is not None and nt == 0

                    pcto_tiles = {}

                    def ring_zero(tile, p0, w0, n, base=None):
                        # zero pad-ring cols 0/97 garbage inside flat [w0, w0+n)
                        # base!=None: tile is chunk-local, cols at base+off-w0
                        for rem in (0, Wp - 1):
                            first = ((w0 - rem + Wp - 1) // Wp) * Wp + rem
                            if first >= w0 + n:
                                continue
                            cnt = (w0 + n - 1 - first) // Wp + 1
                            c0 = first if base is None else base + first - w0
                            v = tile[p0:p0 + 64, c0:c0 + Wp * cnt]
                            v = v.rearrange("p (r w) -> p r w", w=Wp)
                            nc.vector.memset(v[:, :, 0:1], 0.0)

                    def pair_single_group(psum, pw, sw, src, w0, n, start_grp):
                        # 3 K=128 tap-pair matmuls [(dy,0)+(dy,1)] plus 3
                        # K=128 singles [(dy,2); zero rows] — full-K matmuls
                        # only (sub-K matmuls measure ~15% slower here).
                        for dy in range(3):
                            s = w0 + (dy - 1) * Wp - 1
                            nc.tensor.matmul(
                                psum[:, :n], pw[:, dy * 128:(dy + 1) * 128],
                                src[:, s:s + n],
                                start=(start_grp and dy == 0), stop=False)
                        for dy in range(3):
                            s2 = w0 + (dy - 1) * Wp + 1
                            nc.tensor.matmul(
                                psum[:, :n], sw[:, dy * 128:(dy + 1) * 128],
                                src[:, s2:s2 + n],
                                start=False, stop=(dy == 2))

                    def phase_a(k):
                        w0, n = CHUNKS[k]
                        even = (k % 2 == 0)
                        p_if = pool_pif.tile([128, CH], F32, tag="pif")
                        p_cto = pool_pcto.tile([128, CH], F32, tag="pcto")
                        pcto_tiles[k] = p_cto
                        if first_t:
                            # h = c = 0: only the x convs, via [x | x<<1]
                            pair_single_group(p_if, aifT0P, aifT0S, xh, w0, n,
                                              True)
                            pT0 = actoT0PE if even else actoT0PO
                            sT0 = actoT0SE if even else actoT0SO
                            pair_single_group(p_cto, pT0, sT0, xh, w0, n, True)
                        else:
                            for j in range(9):
                                dy, dx = divmod(j, 3)
                                s = w0 + (dy - 1) * Wp + (dx - 1)
                                nc.tensor.matmul(
                                    p_if[:, :n],
                                    aifW[:, j * 128:(j + 1) * 128],
                                    xh[:, s:s + n],
                                    start=(j == 0), stop=False)
                            pair_single_group(p_if, cifPW, cifSW, cur, w0, n,
                                              False)
                            actoW = actoEW if even else actoOW
                            for j in range(9):
                                dy, dx = divmod(j, 3)
                                s = w0 + (dy - 1) * Wp + (dx - 1)
                                nc.tensor.matmul(
                                    p_cto[:, :n],
                                    actoW[:, j * 128:(j + 1) * 128],
                                    xh[:, s:s + n],
                                    start=(j == 0), stop=(j == 8))

                        sl = slice((k % 2) * CH, (k % 2) * CH + n)
                        wsl = slice(w0, w0 + n)
                        ct_in = p_cto[0:64, :n] if even else p_cto[64:128, :n]
                        bc_sl = bcB[0:64] if even else bcB[64:128]
                        nc.scalar.activation(g_if[:, sl], p_if[:, :n], AF.Sigmoid,
                                             bias=bifB[:])
                        nc.scalar.activation(g_ct[:, sl], ct_in, AF.Tanh,
                                             bias=bc_sl)
                        if first_t:
                            # c_new = ct * i   (c_prev == 0)
                            nc.vector.tensor_mul(nxt[0:64, wsl], g_ct[:, sl],
                                                 g_if[0:64, sl])
                        else:
                            # c_new = c*f + ct*i ; read c via the +1-shift dup
                            # half so both inputs share start partition 64
                            nc.vector.tensor_mul(nxt[0:64, wsl],
                                                 cur[64:128, w0 - 1:w0 - 1 + n],
                                                 g_if[64:128, sl])
                            nc.vector.tensor_mul(g_tmp[:, sl], g_ct[:, sl],
                                                 g_if[0:64, sl])
                            nc.vector.tensor_add(nxt[0:64, wsl],
                                                 nxt[0:64, wsl],
                                                 g_tmp[:, sl])
                        # ring-zero this chunk of c_new, then its dup stripe
                        ring_zero(nxt, 0, w0, n)
                        nc.sync.dma_start(nxt[64:128, w0 - 1:w0 - 1 + n],
                                          nxt[0:64, w0:w0 + n])

                    def pb_mm(k, row, wcols, s_off, stop, from_dup, sw=False):
                        # one whc matmul into this chunk's o-half of p_cto
                        w0, n = CHUNKS[k]
                        col = 64 if (k % 2 == 0) else 0
                        p_cto = pcto_tiles[k]
                        s = w0 + s_off - (1 if from_dup else 0)
                        wt = whcSW if sw else whcPW
                        nc.tensor.matmul(
                            p_cto[col:col + 64, :n],
                            wt[:, wcols[0]:wcols[1]], nxt[:, s:s + n],
                            start=False, stop=stop,
                            tile_position=(0, col),
                            skip_group_check=True)

                    def phase_b_post(k):
                        w0, n = CHUNKS[k]
                        even = (k % 2 == 0)
                        col = 64 if even else 0
                        p_cto = pcto_tiles.pop(k)
                        sl = slice((k % 2) * CH, (k % 2) * CH + n)
                        wsl = slice(w0, w0 + n)
                        bo_sl = bobcB[col:col + 64]
                        nc.scalar.activation(g_osb[:, sl], p_cto[col:col + 64, :n],
                                             AF.Sigmoid, bias=bo_sl)
                        nc.scalar.activation(g_th[:, sl], nxt[0:64, wsl], AF.Tanh)
                        if pre_t0:
                            # divert h_new to hout and stream it to DRAM
                            nc.vector.tensor_mul(hout[:, sl], g_osb[:, sl],
                                                 g_th[:, sl])
                            ring_zero(hout, 0, w0, n, base=(k % 2) * CH)
                            nc.sync.dma_start(nsrc[t, :, w0:w0 + n],
                                              hout[:, sl])
                        else:
                            nc.vector.tensor_mul(xh[64:128, wsl], g_osb[:, sl],
                                                 g_th[:, sl])
                            ring_zero(xh, 64, w0, n)

                    def phase_b_pair(ka, kb):
                        ks = [ka] if kb is None else [ka, kb]
                        # K=128 tap-pair matmuls, column-tiled across the pair
                        for dy in range(3):
                            for k in ks:
                                pb_mm(k, -1, (dy * 64, (dy + 1) * 64),
                                      (dy - 1) * Wp - 1, False, False)
                        # K=128 singles [(dy,2); zero rows], col-tiled
                        for dy in range(3):
                            for k in ks:
                                pb_mm(k, -1, (dy * 64, (dy + 1) * 64),
                                      (dy - 1) * Wp + 1, dy == 2, False,
                                      sw=True)
                        for k in ks:
                            phase_b_post(k)

                    def xpiece(lo, hi):
                        nc.sync.dma_start(xh[0:64, lo:hi], nsrc[nt, :, lo:hi])
                        if pre_t0:
                            dlo = max(lo - 1, 0)
                            nc.sync.dma_start(xh[64:128, dlo:hi - 1],
                                              xh[0:64, dlo + 1:hi])

                    for k in range(NCH):
                        phase_a(k)
                        if nsrc is not None and k >= 1:
                            xpiece(PEND[k - 2] if k >= 2 else 0, PEND[k - 1])
                        # pairs run 3 chunks behind phase A so the halo dup
                        # DMA of chunk k-1 has a full chunk to complete --
                        # otherwise the second chunk's matmuls lose their
                        # column-tile concurrency waiting on it
                        if k >= 3 and k % 2 == 1:
                            phase_b_pair(k - 3, k - 2)
                    if nsrc is not None:
                        xpiece(PEND[NCH - 2], ALLOC)
                    phase_b_pair(NCH - 3, NCH - 2)
                    phase_b_pair(NCH - 1, None)
                    if first_t and not pre_t0:
                        # the [x|x<<1] dup left x[W0] at flat 98 (= ring col 0
                        # of row 1, below every chunk window); it must be zero
                        # when this half is next read/saved as h
                        nc.vector.memset(xh[64:128, Wp:Wp + 1], 0.0)

                    if pre_t0:
                        # body pieces went out in phase_b_post; pad head/tail
                        nc.sync.dma_start(nsrc[t, :, 0:W0], zscr[:, 0:W0])
                        tail = CHUNKS[-1][0] + CHUNKS[-1][1]
                        nc.sync.dma_start(nsrc[t, :, tail:ALLOC],
                                          zscr[:, 0:ALLOC - tail])
                    elif l == 0 and n_layers > 1:
                        nc.sync.dma_start(hscr[t, :, :], xh[64:128, :])
                    if l == n_layers - 1:
                        nc.sync.dma_start(hs[t, :, :], xh[64:128, :])
                        nc.sync.dma_start(cs[t, :, :], nxt[0:64, :])
                    step += 1
    nc.compile()
    return nc


def _pack_weights(l, wxi, whi, wci, wxf, whf, wcf, wxc, whc, wxo, who, wco,
                  b_i, b_f, b_c, b_o):
    Z64 = np.zeros((64, 64), np.float32)
    Z128 = np.zeros((64, 128), np.float32)

    def tap(wa, wb, dy, dx):
        # [128(k), 128(m)] block: k = x_cin | h_cin, m = gateA | gateB c_out
        top = np.concatenate([wa[0][l, :, :, dy, dx].T, wb[0][l, :, :, dy, dx].T],
                             axis=1)
        bot = np.concatenate([wa[1][l, :, :, dy, dx].T, wb[1][l, :, :, dy, dx].T],
                             axis=1)
        return np.concatenate([top, bot], axis=0)

    def group(wa, wb):
        return np.concatenate([tap(wa, wb, dy, dx)
                               for dy in range(3) for dx in range(3)], axis=1)

    def m128(wa, wb, dy, dx):
        # x-part-only [64, 128] block
        return np.concatenate([wa[l, :, :, dy, dx].T, wb[l, :, :, dy, dx].T],
                              axis=1)

    def ps_pack(blk, m):
        # pair tiles [(dy,0) ; (dy,1)] for dy=0..2, then singles tile
        # [[ (0,2) | (1,2) | (2,2) ] ; [ 0 | (1,2) | 0 ]]  (m = 64/128 cols)
        z = Z64 if m == 64 else Z128
        pairs = np.concatenate(
            [np.concatenate([blk(dy, 0), blk(dy, 1)], axis=0)
             for dy in range(3)], axis=1)
        singles = np.concatenate([
            np.concatenate([blk(0, 2), z], axis=0),
            np.concatenate([blk(1, 2), z], axis=0),
            np.concatenate([blk(2, 2), z], axis=0)], axis=1)
        return pairs, singles

    cifP, cifS = ps_pack(lambda dy, dx: m128(wci, wcf, dy, dx), 128)
    whcP, whcS = ps_pack(lambda dy, dx: whc[l, :, :, dy, dx].T, 64)
    aifT0P, aifT0S = ps_pack(lambda dy, dx: m128(wxi, wxf, dy, dx), 128)
    actoT0PE, actoT0SE = ps_pack(lambda dy, dx: m128(wxc, wxo, dy, dx), 128)
    actoT0PO, actoT0SO = ps_pack(lambda dy, dx: m128(wxo, wxc, dy, dx), 128)

    def pad128(v):
        # weight dram tensors are declared [128, cols]
        if v.shape[0] == 128:
            return v
        return np.concatenate([v, np.zeros((128 - v.shape[0], v.shape[1]),
                                           np.float32)], axis=0)

    out = {
        f"aif{l}": group((wxi, whi), (wxf, whf)),
        f"cifP{l}": cifP, f"cifS{l}": cifS,
        f"actoE{l}": group((wxc, whc), (wxo, who)),
        f"actoO{l}": group((wxo, who), (wxc, whc)),
        f"whcP{l}": whcP, f"whcS{l}": whcS,
        f"aifT0P{l}": aifT0P, f"aifT0S{l}": aifT0S,
        f"actoT0PE{l}": actoT0PE, f"actoT0SE{l}": actoT0SE,
        f"actoT0PO{l}": actoT0PO, f"actoT0SO{l}": actoT0SO,
        f"bif{l}": np.concatenate([b_i[l], b_f[l]]).reshape(128, 1).astype(np.float32),
        f"bcD{l}": np.concatenate([b_c[l], b_c[l]]).reshape(128, 1).astype(np.float32),
        f"bobcD{l}": np.concatenate([b_o[l] + b_c[l], b_o[l] + b_c[l]]
                                    ).reshape(128, 1).astype(np.float32),
    }
    return {k: (np.ascontiguousarray(v, np.float32) if k.startswith("b")
                else np.ascontiguousarray(pad128(v), np.float16))
            for k, v in out.items()}


def kernel(x, wxi, whi, wci, wxf, whf, wcf, wxc, whc, wxo, who, wco,
           b_i, b_f, b_c, b_o):
    global LAST_EXEC_NS, LAST_RES
    t_steps, bsz = x.shape[0], x.shape[1]
    assert (t_steps, bsz) == (T, B)

    wmaps = {}
    for l in range(L):
        wmaps.update(_pack_weights(l, wxi, whi, wci, wxf, whf, wcf, wxc, whc,
                                   wxo, who, wco, b_i, b_f, b_c, b_o))

    # pad x per batch item: [B, T, C, ALLOC]
    xp = np.zeros((B, T, C, ALLOC), np.float16)
    xview = xp[:, :, :, :NPIX].reshape(B, T, C, Hp, Wp)
    xview[:, :, :, 1:97, 1:97] = np.transpose(x, (1, 0, 2, 3, 4))

    nc = _build(T, L)
    in_maps = [dict(wmaps, xp=np.ascontiguousarray(xp[b])) for b in range(B)]
    import os
    res = run_bass_kernel_spmd(nc, in_maps, core_ids=list(range(B)))
    LAST_EXEC_NS = res.exec_time_ns
    LAST_RES = res

    hs = np.zeros((T, B, C, H, W), np.float32)
    cs = np.zeros((T, B, C, H, W), np.float32)
    for b in range(B):
        hp = res.results[b]["hs"][:, :, :NPIX].astype(np.float32).reshape(T, C, Hp, Wp)
        cp = res.results[b]["cs"][:, :, :NPIX].astype(np.float32).reshape(T, C, Hp, Wp)
        hs[:, b] = hp[:, :, 1:97, 1:97]
        cs[:, b] = cp[:, :, 1:97, 1:97]
    return np.stack([hs, cs])


# revision 6
# speedup vs baseline: 1.1057x; 1.0313x over previous
"""Peephole Conv-LSTM (T=8,B=8,C=64,H=W=96,L=2,K=3) on 8 Trainium2 cores.

Strategy (v2)
-------------
Data-parallel over batch: core b processes batch item b through the whole
T x L recurrence (no collectives).

Conv-as-matmul: images live in SBUF as [C=64 partitions, 98*98 flat padded
pixels].  A 3x3 SAME conv is 9 shifted matmuls accumulating in PSUM, where
the shift is just a flat AP offset into the padded image.

Partition packing (K=128 contraction, M=128 outputs per matmul):
  * xh combo  [x_pad | h_pad]      -> conv(x,Wx*) + conv(h,Wh*) fuse into one
    K=128 matmul per tap; M packs two gates per pass.
  * cc combo  [c_pad | c_pad<<1]   -> peephole convs pair taps (dy,0)+(dy,1)
    in K; (dy,2) singletons are K=128 tiles with zeroed bottom rows.

v2 improvements over baseline:
  * per-chunk x prefetch of the NEXT step's input into xh[0:64] right after
    the last reader chunk of the current step -> no step-boundary PE bubble,
    PE stays HAM-warm (no 8us gaps, no cold-matmul ramps).
  * phase B (conv(c_new, Whc), M=64) pairs two chunks via column tiling
    (tile_position (0,0)/(0,64)) -> 2 concurrent M=64 matmuls, halving
    phase-B PE time.  The o-partial also stays resident in the phase-A
    PSUM tile (acto group continues into the whc matmuls, alternating
    [ct|o]/[o|ct] per chunk parity) -> no opart copy, no extra DVE add.
  * t=0 of each layer skips the peephole matmuls entirely (c_0 = 0) and
    computes c_new = ct*i.

v3 improvements:
  * K=64 "singleton" taps (dx=2 columns of the peephole/whc convs) run as
    row-tiled tile_position pairs: two K=64 matmuls execute concurrently in
    row groups (0,0)/(64,0), the upper group reading the +1-shift dup half.
    cif: 6 -> 5 PE slots; phase-B singles compose row+col tiling across the
    chunk pair: 3 -> 2 PE slots.
  * t=0 of each layer: h and c are zero, so gates need only the x convs.
    An [x | x<<1] dup (built piece-wise during the previous step) lets t=0
    use the same pair+single structure: 10 PE slots instead of 18 per chunk.
    At the step before a layer switch, h_new is written to a scratch tile
    and DMA'd per-piece straight to DRAM, freeing xh[64:128] for the dup.
"""

import numpy as np

import concourse.bass as bass
import concourse.mybir as mybir
from concourse import bacc
from concourse.bass_utils import run_bass_kernel_spmd
from concourse.tile import TileContext

F32 = mybir.dt.float32
F16 = mybir.dt.float16
AF = mybir.ActivationFunctionType

T, B, C, H, W, L, KS = 8, 8, 64, 96, 96, 2, 3
Hp = Wp = 98
NPIX = Hp * Wp              # 9604
ALLOC = NPIX + 1            # slack elem so +1-shifted dup reads stay in bounds
W0 = Wp + 1                 # flat index of output pixel (0,0) in padded coords
NWIN = 96 * Wp + 96 - W0 + 1  # 9406 flat positions spanning all output pixels
CH = 496
CHUNKS = [(W0 + i * CH, min(CH, NWIN - i * CH)) for i in range((NWIN + CH - 1) // CH)]
NCH = len(CHUNKS)
# prefetch piece boundaries: piece k = [PEND[k-1], PEND[k]); its last reader
# in the current step is phase_a(k+1), so it is issued right after that.
PEND = [min(W0 + k * CH + n + Wp + 1, ALLOC) for k, (_, n) in enumerate(CHUNKS)]
PEND[-1] = ALLOC

# row-tiled concurrent K=64 singles (tile_position row 64) — crashes the
# device on this stack (verified in isolation); keep off.
import os as _os
ROWTILE = _os.environ.get("ROWTILE", "0") == "1"

# profiling side-channel for test.py
LAST_EXEC_NS = None
LAST_RES = None


def _build(t_steps: int, n_layers: int, reps: int = 1) -> bass.Bass:
    nc = bacc.Bacc("TRN2", target_bir_lowering=False, debug=False,
                   enable_asserts=False, num_devices=8)

    xp = nc.declare_dram_parameter("xp", [t_steps, C, ALLOC], F16, isOutput=False)
    wd = {}
    wspec = [("aif", 9 * 128), ("cifP", 3 * 128), ("cifS", 3 * 128),
             ("actoE", 9 * 128), ("actoO", 9 * 128),
             ("whcP", 3 * 64), ("whcS", 3 * 64),
             ("aifT0P", 3 * 128), ("aifT0S", 3 * 128),
             ("actoT0PE", 3 * 128), ("actoT0SE", 3 * 128),
             ("actoT0PO", 3 * 128), ("actoT0SO", 3 * 128)]
    for l in range(n_layers):
        for nm, cols in wspec:
            wd[f"{nm}{l}"] = nc.declare_dram_parameter(
                f"{nm}{l}", [128, cols], F16, isOutput=False)
        for nm in ("bif", "bcD", "bobcD"):
            wd[f"{nm}{l}"] = nc.declare_dram_parameter(
                f"{nm}{l}", [128, 1], F32, isOutput=False)
    hs = nc.declare_dram_parameter("hs", [t_steps, C, ALLOC], F16, isOutput=True)
    cs = nc.declare_dram_parameter("cs", [t_steps, C, ALLOC], F16, isOutput=True)
    hscr = nc.dram_tensor("hscr", [t_steps, C, ALLOC], F16) if n_layers > 1 else None

    with TileContext(nc) as tc:
        with (
            tc.tile_pool(name="big", bufs=1) as bigp,
            tc.tile_pool(name="wp", bufs=1) as wp,
            tc.tile_pool(name="gp", bufs=1) as gp,
            tc.tile_pool(name="pif", bufs=2, space="PSUM") as pool_pif,
            tc.tile_pool(name="pcto", bufs=4, space="PSUM") as pool_pcto,
        ):
            xh = bigp.tile([128, ALLOC], F16, name="xh")
            cc0 = bigp.tile([128, ALLOC], F16, name="cc0")
            cc1 = bigp.tile([128, ALLOC], F16, name="cc1")

            wsb = {}
            # load the weights needed first (layer-0 t=0 path) before the rest
            # so the first matmul isn't queued behind ~6 MB of weight DMAs
            prio = ["aifT0P0", "aifT0S0", "actoT0PE0", "actoT0SE0",
                    "actoT0PO0", "actoT0SO0", "bif0", "bcD0", "bobcD0",
                    "whcP0", "whcS0"]
            for key in prio + [k for k in wd if k not in prio]:
                t_ = wd[key]
                tl = wp.tile([128, t_.shape[1]], t_.dtype, name=f"{key}sb")
                nc.sync.dma_start(tl[:], t_[:])
                wsb[key] = tl

            g_if = gp.tile([128, 2 * CH], F16, name="g_if")
            g_ct = gp.tile([64, 2 * CH], F16, name="g_ct")
            g_tmp = gp.tile([64, 2 * CH], F16, name="g_tmp")
            g_th = gp.tile([64, 2 * CH], F16, name="g_th")
            g_osb = gp.tile([64, 2 * CH], F16, name="g_osb")
            hout = gp.tile([64, 2 * CH + Wp], F16, name="hout")
            zscr = gp.tile([64, 128], F16, name="zscr")

            # xh needs no memset: the x DMA covers [0:64] fully and the dup
            # DMA covers [64:128, 0:ALLOC-1]; only the slack col needs zero
            nc.vector.memset(xh[64:128, ALLOC - 1:ALLOC], 0.0)
            nc.vector.memset(cc0[:], 0.0)
            nc.vector.memset(cc1[:], 0.0)
            nc.vector.memset(zscr[:], 0.0)
            nc.sync.dma_start(xh[0:64, :], xp[0, :, :])
            # [x | x<<1] dup for the t=0 pair/single conv structure
            nc.sync.dma_start(xh[64:128, 0:ALLOC - 1], xh[0:64, 1:ALLOC])

            import contextlib
            loop_ctx = tc.For_i(0, reps, 1) if reps > 1 else contextlib.nullcontext()
            step = 0
            with loop_ctx:
             for l in range(n_layers):
                for t in range(t_steps):
                    cur = cc0 if (step % 2 == 0) else cc1
                    nxt = cc1 if (step % 2 == 0) else cc0
                    first_t = (t == 0)
                    aifW = wsb[f"aif{l}"]
                    cifPW, cifSW = wsb[f"cifP{l}"], wsb[f"cifS{l}"]
                    actoEW, actoOW = wsb[f"actoE{l}"], wsb[f"actoO{l}"]
                    whcPW, whcSW = wsb[f"whcP{l}"], wsb[f"whcS{l}"]
                    aifT0P, aifT0S = wsb[f"aifT0P{l}"], wsb[f"aifT0S{l}"]
                    actoT0PE, actoT0SE = wsb[f"actoT0PE{l}"], wsb[f"actoT0SE{l}"]
                    actoT0PO, actoT0SO = wsb[f"actoT0PO{l}"], wsb[f"actoT0SO{l}"]
                    bifB = wsb[f"bif{l}"]
                    bcB = wsb[f"bcD{l}"]
                    bobcB = wsb[f"bobcD{l}"]

                    # source of the NEXT step's x-half (None on the last step)
                    if t + 1 < t_steps:
                        nsrc, nt = (xp if l == 0 else hscr), t + 1
                    elif l + 1 < n_layers:
                        nsrc, nt = hscr, 0
                    else:
                        nsrc, nt = None, 0
                    # next step is a layer's t=0: h_new is diverted to hout
                    # (freeing xh[64:128] for the [x|x<<1] dup of next layer)
                    pre_t0 = nsrc is not None and nt == 0

                    pcto_tiles = {}

                    def ring_zero(tile, p0, w0, n, base=None):
                        # zero pad-ring cols 0/97 garbage inside flat [w0, w0+n)
                        # base!=None: tile is chunk-local, cols at base+off-w0
                        for rem in (0, Wp - 1):
                            first = ((w0 - rem + Wp - 1) // Wp) * Wp + rem
                            if first >= w0 + n:
                                continue
                            cnt = (w0 + n - 1 - first) // Wp + 1
                            c0 = first if base is None else base + first - w0
                            v = tile[p0:p0 + 64, c0:c0 + Wp * cnt]
                            v = v.rearrange("p (r w) -> p r w", w=Wp)
                            nc.vector.memset(v[:, :, 0:1], 0.0)

                    def pair_single_group(psum, pw, sw, src, w0, n, start_grp):
                        # 3 K=128 tap-pair matmuls [(dy,0)+(dy,1)] plus 3
                        # K=128 singles [(dy,2); zero rows] — full-K matmuls
                        # only (sub-K matmuls measure ~15% slower here).
                        for dy in range(3):
                            s = w0 + (dy - 1) * Wp - 1
                            nc.tensor.matmul(
                                psum[:, :n], pw[:, dy * 128:(dy + 1) * 128],
                                src[:, s:s + n],
                                start=(start_grp and dy == 0), stop=False)
                        for dy in range(3):
                            s2 = w0 + (dy - 1) * Wp + 1
                            nc.tensor.matmul(
                                psum[:, :n], sw[:, dy * 128:(dy + 1) * 128],
                                src[:, s2:s2 + n],
                                start=False, stop=(dy == 2))

                    def phase_a(k):
                        w0, n = CHUNKS[k]
                        even = (k % 2 == 0)
                        p_if = pool_pif.tile([128, CH], F32, tag="pif")
                        p_cto = pool_pcto.tile([128, CH], F32, tag="pcto")
                        pcto_tiles[k] = p_cto
                        if first_t:
                            # h = c = 0: only the x convs, via [x | x<<1]
                            pair_single_group(p_if, aifT0P, aifT0S, xh, w0, n,
                                              True)
                            pT0 = actoT0PE if even else actoT0PO
                            sT0 = actoT0SE if even else actoT0SO
                            pair_single_group(p_cto, pT0, sT0, xh, w0, n, True)
                        else:
                            for j in range(9):
                                dy, dx = divmod(j, 3)
                                s = w0 + (dy - 1) * Wp + (dx - 1)
                                nc.tensor.matmul(
                                    p_if[:, :n],
                                    aifW[:, j * 128:(j + 1) * 128],
                                    xh[:, s:s + n],
                                    start=(j == 0), stop=False)
                            pair_single_group(p_if, cifPW, cifSW, cur, w0, n,
                                              False)
                            actoW = actoEW if even else actoOW
                            for j in range(9):
                                dy, dx = divmod(j, 3)
                                s = w0 + (dy - 1) * Wp + (dx - 1)
                                nc.tensor.matmul(
                                    p_cto[:, :n],
                                    actoW[:, j * 128:(j + 1) * 128],
                                    xh[:, s:s + n],
                                    start=(j == 0), stop=(j == 8))

                        sl = slice((k % 2) * CH, (k % 2) * CH + n)
                        wsl = slice(w0, w0 + n)
                        ct_in = p_cto[0:64, :n] if even else p_cto[64:128, :n]
                        bc_sl = bcB[0:64] if even else bcB[64:128]
                        nc.scalar.activation(g_if[:, sl], p_if[:, :n], AF.Sigmoid,
                                             bias=bifB[:])
                        nc.scalar.activation(g_ct[:, sl], ct_in, AF.Tanh,
                                             bias=bc_sl)
                        if first_t:
                            # c_new = ct * i   (c_prev == 0)
                            nc.vector.tensor_mul(nxt[0:64, wsl], g_ct[:, sl],
                                                 g_if[0:64, sl])
                        else:
                            # c_new = c*f + ct*i ; read c via the +1-shift dup
                            # half so both inputs share start partition 64
                            nc.vector.tensor_mul(nxt[0:64, wsl],
                                                 cur[64:128, w0 - 1:w0 - 1 + n],
                                                 g_if[64:128, sl])
                            nc.vector.tensor_mul(g_tmp[:, sl], g_ct[:, sl],
                                                 g_if[0:64, sl])
                            nc.vector.tensor_add(nxt[0:64, wsl],
                                                 nxt[0:64, wsl],
                                                 g_tmp[:, sl])
                        # ring-zero this chunk of c_new, then its dup stripe
                        ring_zero(nxt, 0, w0, n)
                        nc.sync.dma_start(nxt[64:128, w0 - 1:w0 - 1 + n],
                                          nxt[0:64, w0:w0 + n])

                    def pb_mm(k, row, wcols, s_off, stop, from_dup, sw=False):
                        # one whc matmul into this chunk's o-half of p_cto
                        w0, n = CHUNKS[k]
                        col = 64 if (k % 2 == 0) else 0
                        p_cto = pcto_tiles[k]
                        s = w0 + s_off - (1 if from_dup else 0)
                        wt = whcSW if sw else whcPW
                        nc.tensor.matmul(
                            p_cto[col:col + 64, :n],
                            wt[:, wcols[0]:wcols[1]], nxt[:, s:s + n],
                            start=False, stop=stop,
                            tile_position=(0, col),
                            skip_group_check=True)

                    def phase_b_post(k):
                        w0, n = CHUNKS[k]
                        even = (k % 2 == 0)
                        col = 64 if even else 0
                        p_cto = pcto_tiles.pop(k)
                        sl = slice((k % 2) * CH, (k % 2) * CH + n)
                        wsl = slice(w0, w0 + n)
                        bo_sl = bobcB[col:col + 64]
                        nc.scalar.activation(g_osb[:, sl], p_cto[col:col + 64, :n],
                                             AF.Sigmoid, bias=bo_sl)
                        nc.scalar.activation(g_th[:, sl], nxt[0:64, wsl], AF.Tanh)
                        if pre_t0:
                            # divert h_new to hout and stream it to DRAM
                            nc.vector.tensor_mul(hout[:, sl], g_osb[:, sl],
                                                 g_th[:, sl])
                            ring_zero(hout, 0, w0, n, base=(k % 2) * CH)
                            nc.sync.dma_start(nsrc[t, :, w0:w0 + n],
                                              hout[:, sl])
                        else:
                            nc.vector.tensor_mul(xh[64:128, wsl], g_osb[:, sl],
                                                 g_th[:, sl])
                            ring_zero(xh, 64, w0, n)

                    def phase_b_pair(ka, kb):
                        ks = [ka] if kb is None else [ka, kb]
                        # K=128 tap-pair matmuls, column-tiled across the pair
                        for dy in range(3):
                            for k in ks:
                                pb_mm(k, -1, (dy * 64, (dy + 1) * 64),
                                      (dy - 1) * Wp - 1, False, False)
                        # K=128 singles [(dy,2); zero rows], col-tiled
                        for dy in range(3):
                            for k in ks:
                                pb_mm(k, -1, (dy * 64, (dy + 1) * 64),
                                      (dy - 1) * Wp + 1, dy == 2, False,
                                      sw=True)
                        for k in ks:
                            phase_b_post(k)

                    def xpiece(lo, hi):
                        nc.sync.dma_start(xh[0:64, lo:hi], nsrc[nt, :, lo:hi])
                        if pre_t0:
                            dlo = max(lo - 1, 0)
                            nc.sync.dma_start(xh[64:128, dlo:hi - 1],
                                              xh[0:64, dlo + 1:hi])

                    for k in range(NCH):
                        phase_a(k)
                        if nsrc is not None and k >= 1:
                            xpiece(PEND[k - 2] if k >= 2 else 0, PEND[k - 1])
                        # pairs run 3 chunks behind phase A so the halo dup
                        # DMA of chunk k-1 has a full chunk to complete --
                        # otherwise the second chunk's matmuls lose their
                        # column-tile concurrency waiting on it
                        if k >= 3 and k % 2 == 1:
                            phase_b_pair(k - 3, k - 2)
                    if nsrc is not None:
                        xpiece(PEND[NCH - 2], ALLOC)
                    phase_b_pair(NCH - 3, NCH - 2)
                    phase_b_pair(NCH - 1, None)
                    if first_t and not pre_t0:
                        # the [x|x<<1] dup left x[W0] at flat 98 (= ring col 0
                        # of row 1, below every chunk window); it must be zero
                        # when this half is next read/saved as h
                        nc.vector.memset(xh[64:128, Wp:Wp + 1], 0.0)

                    if pre_t0:
                        # body pieces went out in phase_b_post; pad head/tail
                        nc.sync.dma_start(nsrc[t, :, 0:W0], zscr[:, 0:W0])
                        tail = CHUNKS[-1][0] + CHUNKS[-1][1]
                        nc.sync.dma_start(nsrc[t, :, tail:ALLOC],
                                          zscr[:, 0:ALLOC - tail])
                    elif l == 0 and n_layers > 1:
                        nc.sync.dma_start(hscr[t, :, :], xh[64:128, :])
                    if l == n_layers - 1:
                        nc.sync.dma_start(hs[t, :, :], xh[64:128, :])
                        nc.sync.dma_start(cs[t, :, :], nxt[0:64, :])
                    step += 1
    nc.compile()
    return nc


def _pack_weights(l, wxi, whi, wci, wxf, whf, wcf, wxc, whc, wxo, who, wco,
                  b_i, b_f, b_c, b_o):
    Z64 = np.zeros((64, 64), np.float32)
    Z128 = np.zeros((64, 128), np.float32)

    def tap(wa, wb, dy, dx):
        # [128(k), 128(m)] block: k = x_cin | h_cin, m = gateA | gateB c_out
        top = np.concatenate([wa[0][l, :, :, dy, dx].T, wb[0][l, :, :, dy, dx].T],
                             axis=1)
        bot = np.concatenate([wa[1][l, :, :, dy, dx].T, wb[1][l, :, :, dy, dx].T],
                             axis=1)
        return np.concatenate([top, bot], axis=0)

    def group(wa, wb):
        return np.concatenate([tap(wa, wb, dy, dx)
                               for dy in range(3) for dx in range(3)], axis=1)

    def m128(wa, wb, dy, dx):
        # x-part-only [64, 128] block
        return np.concatenate([wa[l, :, :, dy, dx].T, wb[l, :, :, dy, dx].T],
                              axis=1)

    def ps_pack(blk, m):
        # pair tiles [(dy,0) ; (dy,1)] for dy=0..2, then singles tile
        # [[ (0,2) | (1,2) | (2,2) ] ; [ 0 | (1,2) | 0 ]]  (m = 64/128 cols)
        z = Z64 if m == 64 else Z128
        pairs = np.concatenate(
            [np.concatenate([blk(dy, 0), blk(dy, 1)], axis=0)
             for dy in range(3)], axis=1)
        singles = np.concatenate([
            np.concatenate([blk(0, 2), z], axis=0),
            np.concatenate([blk(1, 2), z], axis=0),
            np.concatenate([blk(2, 2), z], axis=0)], axis=1)
        return pairs, singles

    cifP, cifS = ps_pack(lambda dy, dx: m128(wci, wcf, dy, dx), 128)
    whcP, whcS = ps_pack(lambda dy, dx: whc[l, :, :, dy, dx].T, 64)
    aifT0P, aifT0S = ps_pack(lambda dy, dx: m128(wxi, wxf, dy, dx), 128)
    actoT0PE, actoT0SE = ps_pack(lambda dy, dx: m128(wxc, wxo, dy, dx), 128)
    actoT0PO, actoT0SO = ps_pack(lambda dy, dx: m128(wxo, wxc, dy, dx), 128)

    def pad128(v):
        # weight dram tensors are declared [128, cols]
        if v.shape[0] == 128:
            return v
        return np.concatenate([v, np.zeros((128 - v.shape[0], v.shape[1]),
                                           np.float32)], axis=0)

    out = {
        f"aif{l}": group((wxi, whi), (wxf, whf)),
        f"cifP{l}": cifP, f"cifS{l}": cifS,
        f"actoE{l}": group((wxc, whc), (wxo, who)),
        f"actoO{l}": group((wxo, who), (wxc, whc)),
        f"whcP{l}": whcP, f"whcS{l}": whcS,
        f"aifT0P{l}": aifT0P, f"aifT0S{l}": aifT0S,
        f"actoT0PE{l}": actoT0PE, f"actoT0SE{l}": actoT0SE,
        f"actoT0PO{l}": actoT0PO, f"actoT0SO{l}": actoT0SO,
        f"bif{l}": np.concatenate([b_i[l], b_f[l]]).reshape(128, 1).astype(np.float32),
        f"bcD{l}": np.concatenate([b_c[l], b_c[l]]).reshape(128, 1).astype(np.float32),
        f"bobcD{l}": np.concatenate([b_o[l] + b_c[l], b_o[l] + b_c[l]]
                                    ).reshape(128, 1).astype(np.float32),
    }
    return {k: (np.ascontiguousarray(v, np.float32) if k.startswith("b")
                else np.ascontiguousarray(pad128(v), np.float16))
            for k, v in out.items()}


def kernel(x, wxi, whi, wci, wxf, whf, wcf, wxc, whc, wxo, who, wco,
           b_i, b_f, b_c, b_o):
    global LAST_EXEC_NS, LAST_RES
    t_steps, bsz = x.shape[0], x.shape[1]
    assert (t_steps, bsz) == (T, B)

    wmaps = {}
    for l in range(L):
        wmaps.update(_pack_weights(l, wxi, whi, wci, wxf, whf, wcf, wxc, whc,
                                   wxo, who, wco, b_i, b_f, b_c, b_o))

    # pad x per batch item: [B, T, C, ALLOC]
    xp = np.zeros((B, T, C, ALLOC), np.float16)
    xview = xp[:, :, :, :NPIX].reshape(B, T, C, Hp, Wp)
    xview[:, :, :, 1:97, 1:97] = np.transpose(x, (1, 0, 2, 3, 4))

    nc = _build(T, L)
    in_maps = [dict(wmaps, xp=np.ascontiguousarray(xp[b])) for b in range(B)]
    import os
    res = run_bass_kernel_spmd(nc, in_maps, core_ids=list(range(B)))
    LAST_EXEC_NS = res.exec_time_ns
    LAST_RES = res

    hs = np.zeros((T, B, C, H, W), np.float32)
    cs = np.zeros((T, B, C, H, W), np.float32)
    for b in range(B):
        hp = res.results[b]["hs"][:, :, :NPIX].astype(np.float32).reshape(T, C, Hp, Wp)
        cp = res.results[b]["cs"][:, :, :NPIX].astype(np.float32).reshape(T, C, Hp, Wp)
        hs[:, b] = hp[:, :, 1:97, 1:97]
        cs[:, b] = cp[:, :, 1:97, 1:97]
    return np.stack([hs, cs])
